# revision 48
# baseline (speedup 1.0000x reference)
"""Banded sparse attention kernel for Trainium2 (8 NeuronCores, data-parallel over batch).

Problem (per batch element b, S=2048, D=1024, window=20):
    keys_r = keys @ W_reduce.T + b_reduce          [S, D]
    sims   = queries @ keys_r.T + band_mask(S)     [S, S]
    out    = softmax(sims, -1) @ keys              [S, D]

Math used here:
  sims[q,k] = (queries @ W_reduce)[q] . keys[k] + (queries . b_reduce)[q]
  The bias term is constant along k, so it cancels in softmax -> dropped.
  Out-of-band logits get ~-1e8: after the constant -64 shift exp underflows to
  exactly 0.0 in fp32, so softmax over the 168-wide key window equals the
  reference's full-row softmax exactly.

Design (per core, one batch element; all matmuls fp16/bf16 at 1 cycle/row):
  Host packing is layout/dtype only (transpose, reshape, cast -- zero FLOPs):
  Q^T and K^T are shipped pre-transposed fp16 in column-quarters, K row-major
  bf16 for AV values, W pre-arranged fp16 for the stage-B stationary operand.
  - stage B: qw^T = W^T-chunks @ qT, 8dc x 8ec x 4quarters matmuls of N=512,
    PSUM->SBUF fp16 copies alternate ACT/DVE.
  - per q-tile i: 168-wide key window at w0 = clamp(128*i - 20) (the exact
    span of the +-20 band over 128 query rows):
      QK: 8 matmuls N=168 accumulate in PSUM
      mask+shift: DVE scalar_tensor_tensor adds (band_mask - 64) -> SBUF f32
      exp: ACT activation(Exp) with fused row-sum -> bf16 weights
      wT: 2 PE transposes (bf16) -> PSUM -> bf16 SBUF copy
      AV: per 512-half 2 matmuls against shifted bf16 key tiles sk[i] (128
        rows) and the 40-row head of sk[i+1]
      out: PSUM * (1/rowsum) fused into copies -> fp16 -> DMA out
  Scheduling (everything tuned against neuron-profile traces):
  - PE program order B0 B1 T0 B2 T1 B3 T2 T3: stage B runs one quarter ahead
    of the tile group it feeds, giving every DMA ~15us of deadline slack; the
    tensor engine runs stall-free (TRN2 drops to 1.2 GHz after any idle gap
    and needs 3us of continuous execution to reach 2.4 GHz).
  - DMAs ride on sync (HWDGE) + gpsimd (SWDGE, ~100 GB/s); scalar/vector
    issue none because an HWDGE dispatch blocks the engine for roughly the
    transfer time.  The startup-critical 3 MB (W + qt quarter 0) is split
    across both queues; remaining loads are ordered by consumption deadline.
    Output stores ride late on sync after its load tail has drained, one
    per tile (the final tile stores in two halves) so the kernel's last
    DMA is small and starts early.
  Output is fp16 on device, upcast to f32 on host (rounding ~2e-4 rel).
  Measured: 109-114 us HW exec (NTFF; run-to-run HBM contention noise),
  rel err 8.4e-3 (baseline f32r kernel: 157.5 us, 4.2e-3; gate 2e-2).
  Remaining known overheads: ~7us NEFF preamble, ~8us first-load latency,
  ~10us LDWEIGHTS exposure over 480 matmuls, ~4us pipeline drain; PE row
  floor is 78.9us of the ~90us busy.
"""
import numpy as np
import ml_dtypes

B, S, D = 8, 2048, 1024
WINDOW = 20
NEG_BIG = -1e8
NT = S // 128          # 16 q-tiles per core
WN = 168               # key-window width = exact band span
NCORES = 8

_compiled = None

# shifted AV key-tile start rows: sk[j] = K[st : st+128] (sk[16] is 64 rows)
_SK_START = [0] + [j * 128 - 20 for j in range(1, 15)] + [1880, 2008]


def _w0_of(i):
    return 0 if i == 0 else (S - WN if i == NT - 1 else i * 128 - 20)


def _mi_of(i):
    return 0 if i == 0 else (2 if i == NT - 1 else 1)


def _masks_np():
    """3 distinct [128, WN] additive band masks with the -64 exp shift baked in,
    shipped partition-major as [128, 3, WN]."""
    r = np.arange(128)[:, None]
    c = np.arange(WN)[None, :]
    m = np.full((3, 128, WN), NEG_BIG, np.float32)
    for mi, off in enumerate((0, 20, 40)):
        m[mi][np.abs(r + off - c) <= WINDOW] = 0.0
    m -= 64.0
    return np.ascontiguousarray(m.transpose(1, 0, 2))


def _build(dbg=False):
    from contextlib import ExitStack
    import concourse.bass as bass
    import concourse.tile as tile
    from concourse import bacc, mybir

    F32 = mybir.dt.float32
    F16 = mybir.dt.float16
    BF16 = mybir.dt.bfloat16
    EXP = mybir.ActivationFunctionType.Exp
    ADD = mybir.AluOpType.add

    nc = bacc.Bacc("TRN2", target_bir_lowering=False, debug=False,
                   num_devices=NCORES)
    # qt/kt quarters: [q4][p, ec, qq] = X[q4*512+qq, ec*128+p]
    QT = nc.dram_tensor("qt", [4, 128, 8, 512], F16, kind="ExternalInput")
    KT = nc.dram_tensor("kt", [4, 128, 8, 512], F16, kind="ExternalInput")
    KS = nc.dram_tensor("ks", [S, D], BF16, kind="ExternalInput")
    W = nc.dram_tensor("w", [128, 64 * 128], F16, kind="ExternalInput")
    M = nc.dram_tensor("m", [128, 3, WN], F32, kind="ExternalInput")
    I = nc.dram_tensor("i", [128, 128], BF16, kind="ExternalInput")
    O = nc.dram_tensor("o", [S, D], F16, kind="ExternalOutput")
    if dbg:
        DQT = nc.dram_tensor("dqt", [128, 8, S], F16, kind="ExternalOutput")
        DKT = nc.dram_tensor("dkt", [128, 8, S], F16, kind="ExternalOutput")
        DQW = nc.dram_tensor("dqw", [128, 8, S], F16, kind="ExternalOutput")
        DEP = nc.dram_tensor("dep", [NT, 128, WN], F32, kind="ExternalOutput")
        DES = nc.dram_tensor("des", [NT, 128, WN], BF16, kind="ExternalOutput")

    with tile.TileContext(nc) as tc, ExitStack() as ctx:
        def pool(name, bufs, space=bass.MemorySpace.SBUF):
            return ctx.enter_context(tc.tile_pool(name=name, bufs=bufs, space=space))

        const = pool("const", 1)
        p_ep = pool("ep", 2)
        p_es = pool("es", 3)
        p_wt = pool("wt", 2)
        p_out = pool("out", 3)
        p_stat = pool("stat", 4)
        ps_b = pool("ps_b", 2, bass.MemorySpace.PSUM)     # stage B [128,512]
        ps_qk = pool("ps_qk", 2, bass.MemorySpace.PSUM)   # QK [128,WN]
        ps_w = pool("ps_w", 2, bass.MemorySpace.PSUM)     # wT [128,256]
        ps_av = pool("ps_av", 2, bass.MemorySpace.PSUM)   # AV [128,512]

        # alternate PSUM->SBUF copies between ACT and DVE to balance engine load
        _cp = [0]

        def copy(dst, src, scale=None):
            _cp[0] ^= 1
            if scale is not None:
                if _cp[0]:
                    nc.scalar.mul(dst, src, scale)
                else:
                    nc.vector.tensor_scalar_mul(dst, src, scale)
            elif _cp[0]:
                nc.scalar.copy(dst, src)
            else:
                nc.vector.tensor_copy(dst, src)

        qT = const.tile([128, 8, S], F16)       # [p=e%128, ec, q]
        keysT = const.tile([128, 8, S], F16)    # [p=d%128, dc, k]
        qw = const.tile([128, 8, S], F16)       # [p=d%128, dc, q]
        wsb = const.tile([128, 64 * 128], F16)  # [p=e%128, (dc ec)*128+c]
        skA = const.tile([128, 17, D], BF16)    # shifted AV key tiles
        skB2 = const.tile([40, 2, D], BF16)     # B-chunks for tiles 0 and 14
        masks = const.tile([128, 3, WN], F32)
        ident = const.tile([128, 128], BF16)

        # ---- prologue DMAs ----
        # HWDGE dispatch blocks the issuing engine for ~the transfer time, so
        # scalar/vector (which run the softmax + copies) get NO DMAs; sync is
        # ordered so each stage-B quarter's data lands just ahead of use.
        # Startup-critical 3 MB (qt quarter 0 + W) is split across BOTH queues
        # (each streams only ~100-150 GB/s under cross-core HBM contention).
        # qt0 leads on sync so its transfer gets the channels to itself.
        nc.sync.dma_start(qT[:, :, 0:512], QT[0])
        nc.sync.dma_start(wsb[:, 2048:4096], W[:, 2048:4096])
        nc.sync.dma_start(wsb[:, 6144:8192], W[:, 6144:8192])
        nc.sync.dma_start(keysT[:, :, 0:512], KT[0])
        nc.sync.dma_start(masks[:], M[:])
        nc.sync.dma_start(ident[:], I[:])
        nc.sync.dma_start(skA[:, 0, :], KS[0:128, :])
        nc.sync.dma_start(skB2[:, 0, :], KS[128:168, :])
        # sk[1..14] are contiguous rows K[108:1900): strided DMAs by deadline
        nc.sync.dma_start(skA[:, 1:5, :],
                          KS[108:620, :].rearrange("(j p) d -> p j d", p=128))
        nc.sync.dma_start(keysT[:, :, 512:1024], KT[1])
        nc.sync.dma_start(skA[:, 5:8, :],
                          KS[620:1004, :].rearrange("(j p) d -> p j d", p=128))
        nc.sync.dma_start(skA[:, 8:15, :],
                          KS[1004:1900, :].rearrange("(j p) d -> p j d", p=128))
        nc.sync.dma_start(skA[:, 15, :], KS[1880:2008, :])
        nc.sync.dma_start(skA[0:40, 16, :], KS[2008:2048, :])
        nc.sync.dma_start(skB2[:, 1, :], KS[1900:1940, :])
        # gpsimd (SWDGE, ~100 GB/s, otherwise idle): W halves, qt quarters,
        # late keysT
        nc.gpsimd.dma_start(wsb[:, 0:2048], W[:, 0:2048])
        nc.gpsimd.dma_start(wsb[:, 4096:6144], W[:, 4096:6144])
        nc.gpsimd.dma_start(qT[:, :, 512:1024], QT[1])
        nc.gpsimd.dma_start(qT[:, :, 1024:1536], QT[2])
        nc.gpsimd.dma_start(qT[:, :, 1536:2048], QT[3])
        nc.gpsimd.dma_start(keysT[:, :, 1024:1536], KT[2])
        nc.gpsimd.dma_start(keysT[:, :, 1536:2048], KT[3])

        # ---- stage B quarter: qw^T[:, :, q] = W^T @ Q^T[:, :, q] ----
        def stage_B(q4):
            for dc in range(8):
                pb = ps_b.tile([128, 512], F32, name="pb")
                for ec in range(8):
                    nc.tensor.matmul(
                        pb[:],
                        wsb[:, (dc * 8 + ec) * 128:(dc * 8 + ec + 1) * 128],
                        qT[:, ec, q4 * 512:(q4 + 1) * 512],
                        start=(ec == 0), stop=(ec == 7),
                    )
                copy(qw[:, dc, q4 * 512:(q4 + 1) * 512], pb[:])

        # ---- per q-tile: QK -> mask -> exp -> (delayed) wT -> AV -> store ----
        def stage_E(i, es, rs):
            pw = ps_w.tile([128, 256], BF16, name="pw")
            nc.tensor.transpose(pw[:, 0:128], es[:, 0:128], ident[:])
            nc.tensor.transpose(pw[0:40, 128:256], es[:, 128:WN], ident[:])
            wt = p_wt.tile([128, 256], BF16, name="wt")
            copy(wt[:, 0:128], pw[:, 0:128])
            copy(wt[0:40, 128:256], pw[0:40, 128:256])
            if i == 0:
                skb = skB2[:, 0, :]
            elif i == 14:
                skb = skB2[:, 1, :]
            else:
                skb = skA[0:40, i + 1, :]
            osb = p_out.tile([128, D], F16, name="osb")
            for h in range(2):
                po = ps_av.tile([128, 512], F32, name="po")
                nc.tensor.matmul(po[:], wt[:, 0:128],
                                 skA[:, i, h * 512:(h + 1) * 512],
                                 start=True, stop=False)
                nc.tensor.matmul(po[:], wt[0:40, 128:256],
                                 skb[:, h * 512:(h + 1) * 512],
                                 start=False, stop=True)
                copy(osb[:, h * 512:(h + 1) * 512], po[:], scale=rs)
                if i == NT - 1:
                    nc.sync.dma_start(
                        O[i * 128:(i + 1) * 128, h * 512:(h + 1) * 512],
                        osb[:, h * 512:(h + 1) * 512])
            if i != NT - 1:
                nc.sync.dma_start(O[i * 128:(i + 1) * 128, :], osb[:])

        # PE program order: stage B runs one quarter AHEAD of the tile groups
        # it feeds, so every load deadline gains ~15us of slack.
        pend = None
        for step in ("B0", "B1", "T0", "B2", "T1", "B3", "T2", "T3"):
            if step[0] == "B":
                stage_B(int(step[1]))
                continue
            g = int(step[1])
            for i in range(4 * g, 4 * g + 4):
                w0 = _w0_of(i)
                ps = ps_qk.tile([128, WN], F32, name="ps")
                for dc in range(8):
                    nc.tensor.matmul(
                        ps[:],
                        qw[:, dc, i * 128:(i + 1) * 128],
                        keysT[:, dc, w0:w0 + WN],
                        start=(dc == 0), stop=(dc == 7),
                    )
                ep = p_ep.tile([128, WN], F32, name="ep")
                nc.vector.scalar_tensor_tensor(ep[:], ps[:], 0.0,
                                               masks[:, _mi_of(i), :], ADD, ADD)
                es = p_es.tile([128, WN], BF16, name="es")
                ssum = p_stat.tile([128, 1], F32, name="ssum")
                nc.scalar.activation(es[:], ep[:], EXP, accum_out=ssum[:])
                if dbg:
                    nc.gpsimd.dma_start(DEP[i], ep[:])
                    nc.gpsimd.dma_start(DES[i], es[:])
                rs = p_stat.tile([128, 1], F32, name="rs")
                nc.vector.reciprocal(rs[:], ssum[:])
                if pend is not None:
                    stage_E(*pend)
                pend = (i, es, rs)
        stage_E(*pend)

        if dbg:
            nc.gpsimd.dma_start(DQT[:], qT[:])
            nc.gpsimd.dma_start(DKT[:], keysT[:])
            nc.gpsimd.dma_start(DQW[:], qw[:])

    nc.compile()
    return nc


def _in_maps(queries, keys, W_reduce):
    """Host-side packing: dtype casts + layout permutations only (no FLOPs)."""
    w = (np.ascontiguousarray(W_reduce, np.float32)
         .reshape(8, 128, 8, 128).transpose(1, 2, 0, 3)
         .reshape(128, 64 * 128).astype(np.float16))
    masks = _masks_np()
    ident = np.eye(128, dtype=ml_dtypes.bfloat16)
    maps = []
    for c in range(NCORES):
        qc = np.ascontiguousarray(queries[c], np.float32)
        kc = np.ascontiguousarray(keys[c], np.float32)
        maps.append({
            "qt": np.ascontiguousarray(
                qc.reshape(4, 512, 8, 128).transpose(0, 3, 2, 1)
            ).astype(np.float16),
            "kt": np.ascontiguousarray(
                kc.reshape(4, 512, 8, 128).transpose(0, 3, 2, 1)
            ).astype(np.float16),
            "ks": kc.astype(ml_dtypes.bfloat16),
            "w": w,
            "m": masks,
            "i": ident,
        })
    return maps


def kernel(queries, keys, W_reduce, b_reduce):
    """Full-input entry point: shards batch over 8 NeuronCores, returns [B,S,D]."""
    global _compiled
    from concourse.bass_utils import run_bass_kernel_spmd

    if _compiled is None:
        _compiled = _build()
    nc = _compiled

    res = run_bass_kernel_spmd(nc, _in_maps(queries, keys, W_reduce),
                               list(range(NCORES)))
    return np.stack([res.results[c]["o"].astype(np.float32)
                     for c in range(NCORES)])
